# revision 1
# baseline (speedup 1.0000x reference)
"""Trainium2 Bass kernel for nn_DestroyAgent (gnn_message_passing).

Math (algebraically reduced from the reference):
  h0 = coords @ W_embed + b_embed                      [N, 64]
  3 GNN layers: h = relu(h@Wself_l + segsum(h[src]@Wneigh_l, dst) + b_l) + h
  xg = MLP3(h3)[:, 0] per node;  z = MLP3(h0)[:, 0] per node
  out[b, k] = sum_n xg[b, n] * Wp[n] + sum_j z[b, ids[b,k,j]] * Wp[55+j] + bp

Sharding: 8 cores, core c owns graphs [32c, 32c+32) = nodes [1760c, 1760c+1760).
Edges are sharded by dst ownership; per layer, u = h@Wneigh rows are computed
locally and AllGathered so every core can gather u[src] for its own edges.
Segment-sum = dma_gather of u[src] rows + one-hot matmuls per 128-dst block.
"""

import numpy as np
from contextlib import ExitStack

import concourse.bass as bass
import concourse.tile as tile
from concourse import bacc, mybir
from concourse.bass_utils import run_bass_kernel_spmd

dt = mybir.dt
F32 = dt.float32

B, K, D = 256, 256, 3
N_PER = 55
N = B * N_PER            # 14080
E = N * 8                # 112640
EMB = 64
NC_ = 8                  # cores
BPC = B // NC_           # graphs per core = 32
NPC = BPC * N_PER        # nodes per core = 1760
NPAD = 1792              # 14 * 128
NBLK = 14                # local 128-dst blocks
CHUNK = 448              # dense free-dim chunk (4 per core)
GCH = 8                  # gather chunks per layer

_cache = {}


def _preprocess(coords, src, dst, destroy_ids):
    """Host-side index preprocessing -> per-core arrays + global tile schedule."""
    order = np.argsort(dst, kind="stable")
    src_s = src[order].astype(np.int64)
    dst_s = dst[order].astype(np.int64)

    # per-core, per-block edge lists (dst in [c*1760 + 128*b, ...))
    core_block_edges = []
    for c in range(NC_):
        lo_n = c * NPC
        blocks = []
        for b in range(NBLK):
            d0 = lo_n + 128 * b
            d1 = min(lo_n + 128 * (b + 1), lo_n + NPC)
            lo = np.searchsorted(dst_s, d0)
            hi = np.searchsorted(dst_s, d1)
            e_src = src_s[lo:hi]
            e_dst = dst_s[lo:hi]
            o = np.argsort(e_src, kind="stable")   # src-sorted for DMA locality
            blocks.append((e_src[o], e_dst[o] - d0))
        core_block_edges.append(blocks)

    # global per-block tile counts (same program on every core)
    ntb = [max(1, max((len(core_block_edges[c][b][0]) + 127) // 128
                      for c in range(NC_)))
           for b in range(NBLK)]
    T = sum(ntb)

    srcs, dlocs = [], []
    for c in range(NC_):
        s_parts, d_parts = [], []
        for b in range(NBLK):
            es, ed = core_block_edges[c][b]
            pad = ntb[b] * 128 - len(es)
            s_parts.append(np.concatenate([es, np.zeros(pad, np.int64)]))
            d_parts.append(np.concatenate([ed, np.full(pad, -1, np.int64)]))
        srcs.append(np.concatenate(s_parts))
        dlocs.append(np.concatenate(d_parts))
    src_all = np.stack(srcs)                     # [8, T*128]
    dloc_all = np.stack(dlocs)

    # dloc_sb [8, 128, T] fp32: edge t*128+p -> [p, t]
    dloc_sb = dloc_all.reshape(NC_, T, 128).transpose(0, 2, 1).astype(np.float32)
    # src idx wrap: idx i at [i%16, i//16], replicated x8 over partitions
    sw = src_all.reshape(NC_, T * 8, 16).transpose(0, 2, 1).astype(np.int16)
    src_wr = np.tile(sw, (1, 8, 1))              # [8, 128, T*8]

    # destroy ids per core: [128, BPC*2*3] fp32, col = (g*2 + h)*3 + j
    ids = destroy_ids.reshape(NC_, BPC, 2, 128, D)      # [c, g, h, p, j]
    idsf = ids.transpose(0, 3, 1, 2, 4).reshape(NC_, 128, BPC * 2 * D)
    idsf = idsf.astype(np.float32)

    return ntb, T, dloc_sb, src_wr, idsf


def _build(ntb, T, reps=1, stage=99):
    key = (tuple(ntb), T, reps, stage)
    if key in _cache:
        return _cache[key]
    nc = bacc.Bacc("TRN2", target_bir_lowering=False, debug=False, num_devices=NC_)

    def din(name, shape, dtype=F32):
        return nc.dram_tensor(name, list(shape), dtype, kind="ExternalInput").ap()

    coordsT = din("coordsT", [3, NPAD])
    coordsF_d = din("coordsF", [3, N])
    w0p_d = din("w0p", [3, EMB])
    src_wr_d = din("src_wr", [128, T * 8], dt.int16)
    dloc_d = din("dloc", [128, T])
    idsf_d = din("idsf", [128, BPC * 2 * D])
    iota128_d = din("iota128", [128, 128])
    iota55_d = din("iota55", [128, 55])
    ones128_d = din("ones128", [1, 128])
    waug_d = din("waug", [3, EMB])
    wself_d = din("wself", [EMB, 3, EMB])
    wneigh_d = din("wneigh", [EMB, 3, EMB])
    gnnbT_d = din("gnnbT", [EMB, 3])
    w1_d = din("w1", [EMB, 32])
    b1_d = din("b1", [32, 1])
    w2_d = din("w2", [32, 32])
    b2_d = din("b2", [32, 1])
    w3_d = din("w3", [32, 1])
    b3_d = din("b3", [1, 1])
    wpT55_d = din("wpT55", [1, 55])
    wp3_d = din("wp3", [1, 3])
    bp_d = din("bp", [1, 1])

    out_d = nc.dram_tensor("out", [BPC, K], F32, kind="ExternalOutput").ap()

    # DRAM scratch for u rows + allgathered u
    xbs = [[nc.dram_tensor(f"xb{r}_{l}", [NPC, EMB], F32) for l in range(3)]
           for r in range(reps)]
    ybs = [[nc.dram_tensor(f"yb{r}_{l}", [N, EMB], F32, addr_space="Shared")
            for l in range(3)] for r in range(reps)]
    u0s = [nc.dram_tensor(f"u0_{r}", [N, EMB], F32) for r in range(reps)]

    TPC = (T + GCH - 1) // GCH          # tiles per full gather chunk
    CHSZ = [min(TPC, T - g * TPC) for g in range(GCH) if T - g * TPC > 0]
    NCH = len(CHSZ)

    with tile.TileContext(nc) as tc, ExitStack() as ctx:
        sb = ctx.enter_context(tc.tile_pool(name="sb", bufs=1))
        sb2 = ctx.enter_context(tc.tile_pool(name="sb2", bufs=2))
        sb4 = ctx.enter_context(tc.tile_pool(name="sb4", bufs=4))
        sbg = ctx.enter_context(tc.tile_pool(name="sbg", bufs=2))

        def load(name, ap, shape, dtype=F32):
            t = sb.tile(list(shape), dtype, tag=name)
            nc.sync.dma_start(t[:], ap[:])
            return t

        coords_t = load("coords", coordsT, [3, NPAD])
        w0p = load("w0pt", w0p_d, [3, EMB])
        srcw_t = load("srcw", src_wr_d, [128, T * 8], dt.int16)
        dloc_t = load("dloct", dloc_d, [128, T])
        idsf_t = load("idsft", idsf_d, [128, BPC * 2 * D])
        iota128 = load("iota128t", iota128_d, [128, 128])
        iota55 = load("iota55t", iota55_d, [128, 55])
        ones128 = load("ones128t", ones128_d, [1, 128])
        waug = load("waugt", waug_d, [3, EMB])
        wself = load("wselft", wself_d, [EMB, 3, EMB])
        wneigh = load("wneight", wneigh_d, [EMB, 3, EMB])
        gnnbT = load("gnnbTt", gnnbT_d, [EMB, 3])
        w1 = load("w1t", w1_d, [EMB, 32])
        b1c = load("b1t", b1_d, [32, 1])
        w2 = load("w2t", w2_d, [32, 32])
        b2c = load("b2t", b2_d, [32, 1])
        w3 = load("w3t", w3_d, [32, 1])
        b3c = load("b3t", b3_d, [1, 1])
        wpT55 = load("wpT55t", wpT55_d, [1, 55])
        wp3 = load("wp3t", wp3_d, [1, 3])
        bp_t = load("bpt", bp_d, [1, 1])

        # global tile -> block schedule
        block_of = []
        for b in range(NBLK):
            block_of += [b] * ntb[b]
        first_t = {}
        last_t = {}
        for t, b in enumerate(block_of):
            if b not in first_t:
                first_t[b] = t
            last_t[b] = t

        for r in range(reps):
            with tc.tile_pool(name=f"ps{r}", bufs=2, space="PSUM") as ps, \
                 tc.tile_pool(name=f"pb{r}", bufs=3, space="PSUM") as psb, \
                 tc.tile_pool(name=f"pz{r}", bufs=1, space="PSUM") as psz:
                # ---- h0 = Waug.T @ coordsT (bias via ones row) ----
                h0T = sb.tile([EMB, NPAD], F32, tag="h0T")
                for k in range(NPAD // CHUNK):
                    p = ps.tile([EMB, CHUNK], F32, tag="pre")
                    nc.tensor.matmul(p[:], waug[:], coords_t[:, bass.ts(k, CHUNK)],
                                     start=True, stop=True)
                    nc.vector.tensor_copy(h0T[:, bass.ts(k, CHUNK)], p[:])

                def mlp3(srcT, nm):
                    x1 = sb.tile([32, NPAD], F32, tag=f"x1_{nm}")
                    x2 = sb.tile([32, NPAD], F32, tag=f"x2_{nm}")
                    row = sb.tile([1, NPAD], F32, tag=f"row{nm}")
                    for k in range(NPAD // CHUNK):
                        sl = bass.ts(k, CHUNK)
                        p1 = ps.tile([32, CHUNK], F32, tag="u_ps")
                        nc.tensor.matmul(p1[:], w1[:], srcT[:, sl], start=True, stop=True)
                        nc.scalar.activation(x1[:, sl], p1[:],
                                             mybir.ActivationFunctionType.Lrelu,
                                             bias=b1c[:], alpha=0.01)
                        p2 = ps.tile([32, CHUNK], F32, tag="u_ps")
                        nc.tensor.matmul(p2[:], w2[:], x1[:, sl], start=True, stop=True)
                        nc.scalar.activation(x2[:, sl], p2[:],
                                             mybir.ActivationFunctionType.Lrelu,
                                             bias=b2c[:], alpha=0.01)
                        p3 = ps.tile([1, CHUNK], F32, tag="pre")
                        nc.tensor.matmul(p3[:], w3[:], x2[:, sl], start=True, stop=True)
                        nc.scalar.activation(row[:, sl], p3[:],
                                             mybir.ActivationFunctionType.Identity,
                                             bias=b3c[0:1, 0:1])
                    return row

                def emit_layer(l, hcur):
                    hnext = sb.tile([EMB, NPAD], F32, tag=f"hh{1 - (l % 2)}")
                    if l == 0:
                        # u0 = coordsAug @ (Waug @ Wneigh0): local for ALL N
                        yb = u0s[r]
                        NG = 10                      # groups of 11 row-tiles
                        for gi in range(NG):
                            cf = sb2.tile([3, 11 * 128], F32, tag="cf", name=f"cf{r}_{gi}")
                            nc.sync.dma_start(cf[:], coordsF_d[:, bass.ts(gi, 11 * 128)])
                            ust = sb2.tile([128, 11, EMB], F32, tag="ust", name=f"ust{r}_{gi}")
                            for t in range(11):
                                pu = ps.tile([128, EMB], F32, tag="u_ps")
                                nc.tensor.matmul(pu[:], cf[:, bass.ts(t, 128)],
                                                 w0p[:], start=True, stop=True)
                                nc.scalar.copy(ust[:, t, :], pu[:])
                            nc.sync.dma_start(
                                yb.ap()[bass.ts(gi, 11 * 128)]
                                  .rearrange("(t p) f -> p t f", p=128), ust[:])
                    else:
                        # ---- u rows = h @ Wneigh_l (own slice) + AllGather ----
                        u_sb = sb2.tile([128, NBLK, EMB], F32, tag="u_sb")
                        for t in range(NBLK):
                            pu = ps.tile([128, EMB], F32, tag="u_ps")
                            nc.tensor.matmul(pu[:], hcur[:, bass.ts(t, 128)],
                                             wneigh[:, l, :], start=True, stop=True)
                            nc.scalar.copy(u_sb[:, t, :], pu[:])
                        xb, yb = xbs[r][l].ap(), ybs[r][l]
                        nc.sync.dma_start(
                            xb[:1664].rearrange("(t p) f -> p t f", p=128), u_sb[:, :13, :])
                        nc.sync.dma_start(
                            xb[1664:].rearrange("(t p) f -> p t f", p=96), u_sb[:96, 13:14, :])
                        if stage >= 2:
                            nc.gpsimd.collective_compute(
                                "AllGather", mybir.AluOpType.bypass,
                                replica_groups=[list(range(NC_))],
                                ins=[xbs[r][l].ap().opt()], outs=[yb.ap().opt()])

                    # ---- gather u[src] in GCH chunks ----
                    aggT = sb2.tile([EMB, NPAD], F32, tag="aggT")
                    if stage < 4:
                        nc.vector.memset(aggT[:], 0.0)
                    gchunks = []
                    for g in range(NCH):
                        sz = CHSZ[g]
                        gt = sbg.tile([128, sz, EMB], F32, tag="gath",
                                      name=f"gt{r}_{l}_{g}")
                        if stage >= 3:
                            nc.gpsimd.dma_gather(
                                gt[:], yb.ap(),
                                srcw_t[:, g * TPC * 8:(g * TPC + sz) * 8],
                                num_idxs=sz * 128, num_idxs_reg=sz * 128,
                                elem_size=EMB, single_packet=(sz * 128 <= 1024))
                        else:
                            nc.vector.memset(gt[:], 0.0)
                        gchunks.append(gt)
                    # ---- bulk one-hot per chunk, then streamed matmuls ----
                    pb_map = {}
                    if stage >= 4:
                        for g in range(NCH):
                            ohb = sbg.tile([128, CHSZ[g], 128], F32, tag="ohbuf",
                                           name=f"ohb{r}_{l}_{g}")
                            for k in range(CHSZ[g]):
                                t = g * TPC + k
                                nc.vector.tensor_scalar(
                                    ohb[:, k, :], iota128[:], dloc_t[:, t:t + 1],
                                    None, mybir.AluOpType.is_equal)
                            for k in range(CHSZ[g]):
                                t = g * TPC + k
                                b = block_of[t]
                                if first_t[b] == t:
                                    pb_map[b] = psb.tile([EMB, 128], F32, tag="blk", name=f"pbm{r}_{l}_{b}")
                                nc.tensor.matmul(pb_map[b][:], gchunks[g][:, k, :],
                                                 ohb[:, k, :],
                                                 start=(first_t[b] == t),
                                                 stop=(last_t[b] == t))
                                if last_t[b] == t:
                                    nc.scalar.copy(aggT[:, bass.ts(b, 128)],
                                                   pb_map.pop(b)[:])

                    # ---- dense update ----
                    for k in range(NPAD // CHUNK):
                        sl = bass.ts(k, CHUNK)
                        p = ps.tile([EMB, CHUNK], F32, tag="pre")
                        nc.tensor.matmul(p[:], wself[:, l, :], hcur[:, sl],
                                         start=True, stop=True)
                        nc.vector.tensor_add(p[:], p[:], aggT[:, sl])
                        relu = sb2.tile([EMB, CHUNK], F32, tag="relu")
                        nc.scalar.activation(relu[:], p[:],
                                             mybir.ActivationFunctionType.Relu,
                                             bias=gnnbT[:, l:l + 1])
                        nc.vector.tensor_add(hnext[:, sl], relu[:], hcur[:, sl])
                    return hnext

                h1T = emit_layer(0, h0T)

                # ======== tail part A: depends only on h0T; fills AG/gather idle ========
                stg = sb.tile([128, BPC * 2], F32, tag="stage")
                if stage >= 5:
                    zrow = mlp3(h0T, "z")
                    zw_all = sb.tile([1, BPC * 3 * 55], F32, tag="zw")
                    z_v = (zrow[0:1, 0:NPC].rearrange("o (g n) -> o g n", n=55)
                           .unsqueeze(2).broadcast_to([1, BPC, 3, 55]))
                    wp3_v = (wp3[0:1, :].unsqueeze(1).unsqueeze(3)
                             .broadcast_to([1, BPC, 3, 55]))
                    nc.vector.tensor_tensor(
                        zw_all[0:1, :].rearrange("o (g j n) -> o g j n", j=3, n=55),
                        z_v, wp3_v, mybir.AluOpType.mult)
                    if stage < 6:
                        nc.vector.memset(stg[:], 0.0)
                    for g in range(BPC if stage >= 6 else 0):
                        zw_ps = psz.tile([128, 165], F32, tag="zwps")
                        nc.tensor.matmul(zw_ps[:], ones128[:],
                                         zw_all[0:1, g * 165:(g + 1) * 165],
                                         start=True, stop=True)
                        for h in range(2):
                            col = (g * 2 + h) * 3
                            oh3 = sb4.tile([128, 3, 55], F32, tag="oh3")
                            for j in range(3):
                                nc.vector.tensor_scalar(oh3[:, j, :], iota55[:],
                                                        idsf_t[:, col + j:col + j + 1],
                                                        None, mybir.AluOpType.is_equal)
                            scr = sb2.tile([128, 165], F32, tag="scr")
                            nc.vector.tensor_tensor(
                                scr[:], oh3[:].rearrange("p a b -> p (a b)"),
                                zw_ps[:], mybir.AluOpType.mult)
                            nc.vector.tensor_reduce(
                                stg[:, g * 2 + h:g * 2 + h + 1],
                                scr[:].rearrange("p (a b) -> p a b", b=165),
                                axis=mybir.AxisListType.XY, op=mybir.AluOpType.add)

                h2T = emit_layer(1, h1T)
                h3T = emit_layer(2, h2T)

                # ======== tail part B: xg from h3 ========
                if stage >= 5:
                    xgrow = mlp3(h3T, "xg")
                    wpx = sb.tile([1, NPC], F32, tag="wpx")
                    xg_v = xgrow[0:1, 0:NPC].rearrange("o (g n) -> o g n", n=55)
                    wp_v = wpT55[0:1, :].unsqueeze(1).broadcast_to([1, BPC, 55])
                    nc.vector.tensor_tensor(
                        wpx[0:1, :].rearrange("o (g n) -> o g n", n=55),
                        xg_v, wp_v, mybir.AluOpType.mult)
                    s_row = sb.tile([1, BPC], F32, tag="srow")
                    nc.vector.tensor_reduce(
                        s_row[:], wpx[0:1, :].rearrange("o (g n) -> o g n", n=55),
                        axis=mybir.AxisListType.X, op=mybir.AluOpType.add)
                    nc.vector.tensor_scalar(s_row[:], s_row[:], bp_t[0:1, 0:1], None,
                                            mybir.AluOpType.add)
                    sg_ps = ps.tile([128, BPC], F32, tag="pre")
                    nc.tensor.matmul(sg_ps[:], ones128[:], s_row[:], start=True, stop=True)
                    outb = sb.tile([128, BPC * 2], F32, tag="outb")
                    nc.vector.tensor_tensor(
                        outb[:].rearrange("p (g h) -> p g h", h=2),
                        stg[:].rearrange("p (g h) -> p g h", h=2),
                        sg_ps[:].unsqueeze(2).broadcast_to([128, BPC, 2]),
                        mybir.AluOpType.add)
                else:
                    outb = sb.tile([128, BPC * 2], F32, tag="outb")
                    nc.vector.memset(outb[:], 0.0)
                nc.sync.dma_start(
                    out_d.rearrange("g (h p) -> p g h", p=128),
                    outb[:].rearrange("p (g h) -> p g h", h=2))

    nc.finalize()
    _cache[key] = nc
    return nc


def kernel(**inputs):
    coords = np.asarray(inputs["coords"], np.float32)
    src = np.asarray(inputs["src"])
    dst = np.asarray(inputs["dst"])
    destroy_ids = np.asarray(inputs["destroy_ids"])
    W_embed = np.asarray(inputs["W_embed"], np.float32)
    b_embed = np.asarray(inputs["b_embed"], np.float32)
    Wself = np.asarray(inputs["Wself"], np.float32)
    Wneigh = np.asarray(inputs["Wneigh"], np.float32)
    gnn_b = np.asarray(inputs["gnn_b"], np.float32)
    W1 = np.asarray(inputs["W1"], np.float32)
    b1 = np.asarray(inputs["b1"], np.float32)
    W2 = np.asarray(inputs["W2"], np.float32)
    b2 = np.asarray(inputs["b2"], np.float32)
    W3 = np.asarray(inputs["W3"], np.float32)
    b3 = np.asarray(inputs["b3"], np.float32)
    Wp = np.asarray(inputs["Wp"], np.float32)
    bp = np.asarray(inputs["bp"], np.float32)

    ntb, T, dloc_sb, src_wr, idsf = _preprocess(coords, src, dst, destroy_ids)
    nc = _build(ntb, T, reps=1)

    waug = np.concatenate([W_embed, b_embed[None, :]], 0)          # [3, 64]
    coordsF_full = np.concatenate([coords.T, np.ones((1, N), np.float32)], 0)
    w0p_np = (waug @ Wneigh[0]).astype(np.float32)
    iota128 = np.tile(np.arange(128, dtype=np.float32), (128, 1))
    iota55 = np.tile(np.arange(55, dtype=np.float32), (128, 1))
    ones128 = np.ones((1, 128), np.float32)

    in_maps = []
    for c in range(NC_):
        cs = coords[c * NPC:(c + 1) * NPC]                          # [1760, 2]
        coordsT = np.zeros((3, NPAD), np.float32)
        coordsT[:2, :NPC] = cs.T
        coordsT[2, :NPC] = 1.0
        in_maps.append({
            "coordsT": coordsT,
            "coordsF": coordsF_full, "w0p": w0p_np,
            "src_wr": src_wr[c],
            "dloc": dloc_sb[c],
            "idsf": idsf[c],
            "iota128": iota128, "iota55": iota55, "ones128": ones128,
            "waug": waug,
            "wself": np.ascontiguousarray(Wself.transpose(1, 0, 2)),
            "wneigh": np.ascontiguousarray(Wneigh.transpose(1, 0, 2)),
            "gnnbT": gnn_b.T.copy(),
            "w1": W1, "b1": b1[:, None], "w2": W2, "b2": b2[:, None],
            "w3": W3, "b3": b3[:, None],
            "wpT55": Wp[:55, 0][None, :].copy(), "wp3": Wp[55:, 0][None, :].copy(),
            "bp": bp[:, None],
        })

    global _last_in_maps
    _last_in_maps = in_maps
    res = run_bass_kernel_spmd(nc, in_maps, core_ids=list(range(NC_)))
    return np.concatenate([res.results[c]["out"] for c in range(NC_)], 0)



# revision 13
# speedup vs baseline: 1.9346x; 1.9346x over previous
"""Trainium2 Bass kernel for nn_DestroyAgent (gnn_message_passing).

Math (algebraically reduced from the reference):
  h0 = coords @ W_embed + b_embed                      [N, 64]
  3 GNN layers: h = relu(h@Wself_l + segsum(h[src]@Wneigh_l, dst) + b_l) + h
  xg = MLP3(h3)[:, 0] per node;  z = MLP3(h0)[:, 0] per node
  out[b, k] = sum_n xg[b, n] * Wp[n] + sum_j z[b, ids[b,k,j]] * Wp[55+j] + bp

Sharding: 8 cores, core c owns graphs [32c, 32c+32) = nodes [1760c, 1760c+1760).
Edges are sharded by dst ownership, tiled into 128-edge tiles per 128-dst
block; aggregation = one-hot matmuls per tile (edge-contraction on PE).

Key structure vs the naive version:
- Layer 0 needs no gather/collective: u0[src] = coordsAug[src] @ (Waug@Wneigh0)
  and coordsAug[src] is host-gathered (static edge list), so the per-block
  aggregate is coordsET-tile @ onehot (PE) followed by a tiny [3,64] matmul.
- Layers 1-2: u = h@Wneigh rows are written to DRAM, AllGathered, then
  dma_gather'd per 128-edge tile (f32 256B rows), cast to bf16 for the
  one-hot matmuls.
- The destroy-set tail is a host-built matrix M2 [110, 128] per (graph, half):
  rows 0:55 = sum_j wp3[j]*onehot(ids), rows 55:110 = wp55; out column =
  M2^T @ concat(z_g, xg_g). 64 tiny matmuls replace the serial DVE loop.
- One-hot dst matrices are static across layers and reps: built once, bf16.
- Dense/MLP matmuls in float32r (1 cyc/row at free>=256 vs 4 for fp32).
"""

import numpy as np
from contextlib import ExitStack

import ml_dtypes
import concourse.bass as bass
import concourse.tile as tile
from concourse import bacc, mybir
from concourse.bass_utils import run_bass_kernel_spmd

dt = mybir.dt
F32 = dt.float32
F32R = dt.float32r
BF16 = dt.bfloat16
NPBF = ml_dtypes.bfloat16

B, K, D = 256, 256, 3
N_PER = 55
N = B * N_PER            # 14080
E = N * 8                # 112640
EMB = 64
NC_ = 8                  # cores
BPC = B // NC_           # graphs per core = 32
NPC = BPC * N_PER        # nodes per core = 1760
NPAD = 1792              # 14 * 128
NBLK = 14                # local 128-dst blocks
CHUNK = 448              # dense free-dim chunk (4 per core)
GCH = 8                  # gather chunks per layer

_cache = {}


def _preprocess(coords, src, dst, destroy_ids, W_embed=None, b_embed=None,
                Wneigh=None, Wp=None, bp=None):
    """Host-side index/static preprocessing -> per-core arrays + schedule."""
    order = np.argsort(dst, kind="stable")
    src_s = src[order].astype(np.int64)
    dst_s = dst[order].astype(np.int64)

    core_block_edges = []
    for c in range(NC_):
        lo_n = c * NPC
        blocks = []
        for b in range(NBLK):
            d0 = lo_n + 128 * b
            d1 = min(lo_n + 128 * (b + 1), lo_n + NPC)
            lo = np.searchsorted(dst_s, d0)
            hi = np.searchsorted(dst_s, d1)
            e_src = src_s[lo:hi]
            e_dst = dst_s[lo:hi]
            o = np.argsort(e_src, kind="stable")   # src-sorted for DMA locality
            blocks.append((e_src[o], e_dst[o] - d0))
        core_block_edges.append(blocks)

    ntb = [max(1, max((len(core_block_edges[c][b][0]) + 127) // 128
                      for c in range(NC_)))
           for b in range(NBLK)]
    T = sum(ntb)

    srcs, dlocs = [], []
    for c in range(NC_):
        s_parts, d_parts = [], []
        for b in range(NBLK):
            es, ed = core_block_edges[c][b]
            pad = ntb[b] * 128 - len(es)
            s_parts.append(np.concatenate([es, np.zeros(pad, np.int64)]))
            d_parts.append(np.concatenate([ed, np.full(pad, -1, np.int64)]))
        srcs.append(np.concatenate(s_parts))
        dlocs.append(np.concatenate(d_parts))
    src_all = np.stack(srcs)                     # [8, T*128]
    dloc_all = np.stack(dlocs)

    # dloc_sb [8, 128, T] fp32: edge t*128+p -> [p, t]
    dloc_sb = dloc_all.reshape(NC_, T, 128).transpose(0, 2, 1).astype(np.float32)
    # src idx wrap: idx i at [i%16, i//16], replicated x8 over partitions
    sw = src_all.reshape(NC_, T * 8, 16).transpose(0, 2, 1).astype(np.int16)
    src_wr = np.tile(sw, (1, 8, 1))              # [8, 128, T*8]

    extras = None
    if W_embed is not None:
        # per-edge augmented coords, tile-slot order: [8, 128(e), T*3] bf16
        caug = np.concatenate([coords, np.ones((N, 1), np.float32)], 1)  # [N,3]
        ce = caug[src_all]                                   # [8, T*128, 3]
        ce[dloc_all < 0] = 0.0
        coordsET = (ce.reshape(NC_, T, 128, 3).transpose(0, 2, 1, 3)
                    .reshape(NC_, 128, T * 3).astype(NPBF))

        # destroy matrix M2 [8, 110, BPC*2*128] bf16
        wp55 = Wp[:55, 0].astype(np.float32)
        wp3 = Wp[55:, 0].astype(np.float32)
        ids8 = destroy_ids.reshape(NC_, BPC, 2, 128, D)
        m2 = np.zeros((NC_, BPC, 2, 128, 111), np.float32)
        for j in range(D):
            np.add.at(m2, (np.arange(NC_)[:, None, None, None],
                           np.arange(BPC)[None, :, None, None],
                           np.arange(2)[None, None, :, None],
                           np.arange(128)[None, None, None, :],
                           ids8[..., j]), wp3[j])
        m2[..., 55:110] = wp55
        m2[..., 110] = float(bp[0])
        m2 = (m2.transpose(0, 4, 1, 2, 3).reshape(NC_, 111, BPC * 2 * 128)
              .astype(NPBF))
        extras = (coordsET, m2)

    return ntb, T, dloc_sb, src_wr, extras


def _build(ntb, T, reps=1, stage=99, nq=1):
    key = (tuple(ntb), T, reps, stage, nq)
    if key in _cache:
        return _cache[key]
    nc = bacc.Bacc("TRN2", target_bir_lowering=False, debug=False,
                   num_devices=NC_, num_swdge_queues=nq)

    def din(name, shape, dtype=F32):
        return nc.dram_tensor(name, list(shape), dtype, kind="ExternalInput").ap()

    coordsT_d = din("coordsT", [3, NPAD])
    coordsET_d = din("coordsET", [128, T * 3], BF16)
    w0p_d = din("w0p", [3, EMB], BF16)
    src_wr_d = din("src_wr", [128, T * 8], dt.int16)
    dloc_d = din("dloc", [128, T])
    m2_d = din("m2", [111, BPC * 2 * 128], BF16)
    iota128_d = din("iota128", [128, 128])
    waug_d = din("waug", [3, EMB])
    wself_d = din("wself", [EMB, 3, EMB])
    wneigh_d = din("wneigh", [EMB, 3, EMB])
    gnnbT_d = din("gnnbT", [EMB, 3])
    w1_d = din("w1", [EMB, 32])
    b1_d = din("b1", [32, 1])
    w2_d = din("w2", [32, 32])
    b2_d = din("b2", [32, 1])
    w3_d = din("w3", [32, 1])
    b3_d = din("b3", [1, 1])
    bp_d = din("bp", [1, 1])

    out_d = nc.dram_tensor("out", [BPC, K], F32, kind="ExternalOutput").ap()

    # DRAM scratch: u rows (local slice) + allgathered u, per rep, layers 1-2
    xbs = [[nc.dram_tensor(f"xb{r}_{l}", [NPC, EMB], F32) for l in range(2)]
           for r in range(reps)]
    ybs = [[nc.dram_tensor(f"yb{r}_{l}", [N, EMB], F32, addr_space="Shared")
            for l in range(2)] for r in range(reps)]
    # z/xg row bounce buffers (node-major -> [55, graphs] transpose via DRAM)
    zds = [nc.dram_tensor(f"zd{r}", [2, NPC], F32) for r in range(reps)]

    TPC = (T + GCH - 1) // GCH
    CHSZ = [min(TPC, T - g * TPC) for g in range(GCH) if T - g * TPC > 0]
    NCH = len(CHSZ)

    # global tile -> block schedule
    block_of = []
    for b in range(NBLK):
        block_of += [b] * ntb[b]
    first_t = {}
    last_t = {}
    for t, b in enumerate(block_of):
        if b not in first_t:
            first_t[b] = t
        last_t[b] = t

    with tile.TileContext(nc) as tc, ExitStack() as ctx:
        sb = ctx.enter_context(tc.tile_pool(name="sb", bufs=1))
        sb2 = ctx.enter_context(tc.tile_pool(name="sb2", bufs=2))
        sbg = ctx.enter_context(tc.tile_pool(name="sbg", bufs=2))
        ps = ctx.enter_context(tc.tile_pool(name="ps", bufs=2, space="PSUM"))
        psb = ctx.enter_context(tc.tile_pool(name="psb", bufs=3, space="PSUM"))
        psu = ctx.enter_context(tc.tile_pool(name="psu", bufs=2, space="PSUM"))

        def load(name, ap, shape, dtype=F32):
            t = sb.tile(list(shape), dtype, tag=name)
            nc.sync.dma_start(t[:], ap[:])
            return t

        coords_t = load("coordsT", coordsT_d, [3, NPAD])
        coordsET = load("coordsET", coordsET_d, [128, T * 3], BF16)
        w0p = load("w0p", w0p_d, [3, EMB], BF16)
        srcw_t = load("srcw", src_wr_d, [128, T * 8], dt.int16)
        dloc_t = load("dloct", dloc_d, [128, T])
        m2_t = load("m2", m2_d, [111, BPC * 2 * 128], BF16)
        iota128 = load("iota128t", iota128_d, [128, 128])
        waug = load("waugt", waug_d, [3, EMB])
        wself = load("wselft", wself_d, [EMB, 3, EMB])
        wneigh = load("wneight", wneigh_d, [EMB, 3, EMB])
        gnnbT = load("gnnbTt", gnnbT_d, [EMB, 3])
        w1 = load("w1t", w1_d, [EMB, 32])
        b1c = load("b1t", b1_d, [32, 1])
        w2 = load("w2t", w2_d, [32, 32])
        b2c = load("b2t", b2_d, [32, 1])
        w3 = load("w3t", w3_d, [32, 1])
        b3c = load("b3t", b3_d, [1, 1])
        bp_t = load("bpt", bp_d, [1, 1])

        # one-hot dst matrices: static across layers AND reps; bf16
        ohb = sb.tile([128, T, 128], BF16, tag="ohb")
        for t in range(T):
            nc.vector.tensor_scalar(ohb[:, t, :], iota128[:],
                                    dloc_t[:, t:t + 1], None,
                                    mybir.AluOpType.is_equal)

        def r32(apv):
            return apv

        def mlp3(srcT, nm):
            """64->32->32->1 MLP over all NPAD columns; returns [1, NPAD]."""
            row = sb2.tile([1, NPAD], F32, tag="mlprow", name=f"row_{nm}")
            for k in range(NPAD // CHUNK):
                sl = bass.ts(k, CHUNK)
                x1 = sb2.tile([32, CHUNK], F32, tag="mx1", name=f"x1_{nm}_{k}")
                x2 = sb2.tile([32, CHUNK], F32, tag="mx2", name=f"x2_{nm}_{k}")
                p1 = ps.tile([32, CHUNK], F32, tag="pre", name=f"p1_{nm}_{k}")
                nc.tensor.matmul(p1[:], r32(w1[:]), r32(srcT[:, sl]),
                                 start=True, stop=True)
                nc.scalar.activation(x1[:], p1[:],
                                     mybir.ActivationFunctionType.Lrelu,
                                     bias=b1c[:], alpha=0.01)
                p2 = ps.tile([32, CHUNK], F32, tag="pre", name=f"p2_{nm}_{k}")
                nc.tensor.matmul(p2[:], r32(w2[:]), r32(x1[:]),
                                 start=True, stop=True)
                nc.scalar.activation(x2[:], p2[:],
                                     mybir.ActivationFunctionType.Lrelu,
                                     bias=b2c[:], alpha=0.01)
                p3 = ps.tile([1, CHUNK], F32, tag="pre", name=f"p3_{nm}_{k}")
                nc.tensor.matmul(p3[:], r32(w3[:]), r32(x2[:]),
                                 start=True, stop=True)
                nc.scalar.activation(row[:, sl], p3[:],
                                     mybir.ActivationFunctionType.Identity,
                                     bias=b3c[0:1, 0:1])
            return row

        for r in range(reps):
            # ---- h0 = Waug.T @ coordsT ----
            h0T = sb2.tile([EMB, NPAD], F32, tag="h0T", name=f"h0T_{r}")
            for k in range(NPAD // CHUNK):
                p = ps.tile([EMB, CHUNK], F32, tag="pre", name=f"ph0_{r}_{k}")
                nc.tensor.matmul(p[:], r32(waug[:]),
                                 r32(coords_t[:, bass.ts(k, CHUNK)]),
                                 start=True, stop=True)
                nc.vector.tensor_copy(h0T[:, bass.ts(k, CHUNK)], p[:])

            def dense_update(l, hcur, aggT, nm):
                hnext = sb2.tile([EMB, NPAD], F32, tag=f"hh{l % 2}",
                                 name=f"h{l + 1}T_{r}")
                for k in range(NPAD // CHUNK):
                    sl = bass.ts(k, CHUNK)
                    p = ps.tile([EMB, CHUNK], F32, tag="pre",
                                name=f"pd_{nm}_{k}")
                    nc.tensor.matmul(p[:], r32(wself[:, l, :]),
                                     r32(hcur[:, sl]), start=True, stop=True)
                    nc.vector.tensor_add(p[:], p[:], aggT[:, sl])
                    relu = sb2.tile([EMB, CHUNK], F32, tag="relu",
                                    name=f"relu_{nm}_{k}")
                    nc.scalar.activation(relu[:], p[:],
                                         mybir.ActivationFunctionType.Relu,
                                         bias=gnnbT[:, l:l + 1])
                    nc.vector.tensor_add(hnext[:, sl], relu[:], hcur[:, sl])
                return hnext

            # ======== layer 0: agg from host-gathered edge coords ========
            agg0T = sb2.tile([EMB, NPAD], F32, tag="aggT", name=f"agg0T_{r}")
            if stage >= 4:
                cb_map = {}
                for t in range(T):
                    b = block_of[t]
                    if first_t[b] == t:
                        cb_map[b] = psu.tile([3, 128], F32, tag="u_ps",
                                             name=f"cb_{r}_{b}")
                    nc.tensor.matmul(cb_map[b][:],
                                     coordsET[:, t * 3:(t + 1) * 3],
                                     ohb[:, t, :],
                                     start=(first_t[b] == t),
                                     stop=(last_t[b] == t))
                    if last_t[b] == t:
                        cbs = sb2.tile([3, 128], BF16, tag="cbs",
                                       name=f"cbs_{r}_{b}")
                        nc.scalar.copy(cbs[:], cb_map.pop(b)[:])
                        pa = psu.tile([EMB, 128], F32, tag="u_ps",
                                      name=f"pa0_{r}_{b}")
                        nc.tensor.matmul(pa[:], w0p[:], cbs[:],
                                         start=True, stop=True)
                        nc.scalar.copy(agg0T[:, bass.ts(b, 128)], pa[:])
            else:
                nc.vector.memset(agg0T[:], 0.0)
            h1T = dense_update(0, h0T, agg0T, f"d0_{r}")

            def u_and_allgather(l, hcur):
                # u rows = h @ Wneigh_l (own slice), DMA out, AllGather
                u_sb = sb2.tile([128, NBLK, EMB], F32, tag="u_sb",
                                name=f"u_sb{r}_{l}")
                for t in range(NBLK):
                    pu = psu.tile([128, EMB], F32, tag="u_ps",
                                  name=f"pu{r}_{l}_{t}")
                    nc.tensor.matmul(pu[:], hcur[:, bass.ts(t, 128)],
                                     wneigh[:, l, :], start=True, stop=True)
                    nc.scalar.copy(u_sb[:, t, :], pu[:])
                xb, yb = xbs[r][l - 1].ap(), ybs[r][l - 1]
                nc.sync.dma_start(
                    xb[:1664].rearrange("(t p) f -> p t f", p=128),
                    u_sb[:, :13, :])
                nc.sync.dma_start(
                    xb[1664:].rearrange("(t p) f -> p t f", p=96),
                    u_sb[:96, 13:14, :])
                if stage >= 2:
                    nc.gpsimd.collective_compute(
                        "AllGather", mybir.AluOpType.bypass,
                        replica_groups=[list(range(NC_))],
                        ins=[xbs[r][l - 1].ap().opt()], outs=[yb.ap().opt()])
                return yb

            def gather_and_agg(l, yb, nm):
                aggT = sb2.tile([EMB, NPAD], F32, tag="aggT",
                                name=f"agg{l}T_{r}")
                if stage < 4:
                    nc.vector.memset(aggT[:], 0.0)
                pb_map = {}
                for g in range(NCH):
                    sz = CHSZ[g]
                    gt = sbg.tile([128, TPC, EMB], F32, tag="gath",
                                  name=f"gt{r}_{l}_{g}")
                    if stage >= 3:
                        nc.gpsimd.dma_gather(
                            gt[:, :sz, :], yb.ap(),
                            srcw_t[:, g * TPC * 8:(g * TPC + sz) * 8],
                            num_idxs=sz * 128, num_idxs_reg=sz * 128,
                            elem_size=EMB, queue_num=g % nq,
                            single_packet=(sz * 128 <= 1024))
                    else:
                        nc.vector.memset(gt[:], 0.0)
                    gtb = sbg.tile([128, TPC, EMB], BF16, tag="gathb",
                                   name=f"gtb{r}_{l}_{g}")
                    nc.vector.tensor_copy(gtb[:, :sz, :], gt[:, :sz, :])
                    if stage >= 4:
                        for k in range(sz):
                            t = g * TPC + k
                            b = block_of[t]
                            if first_t[b] == t:
                                pb_map[b] = psb.tile(
                                    [EMB, 128], F32, tag="blk",
                                    name=f"pbm{r}_{l}_{b}")
                            nc.tensor.matmul(pb_map[b][:], gtb[:, k, :],
                                             ohb[:, t, :],
                                             start=(first_t[b] == t),
                                             stop=(last_t[b] == t))
                            if last_t[b] == t:
                                nc.scalar.copy(aggT[:, bass.ts(b, 128)],
                                               pb_map.pop(b)[:])
                return aggT

            # ======== layer 1 ========
            yb1 = u_and_allgather(1, h1T)

            # tail part A (z-mlp from h0) fills the AllGather window
            v_t = sb2.tile([111, BPC], F32, tag="v", name=f"v_{r}")
            nc.vector.memset(v_t[:], 1.0)
            zd = zds[r].ap()
            if stage >= 5:
                zrow = mlp3(h0T, f"z{r}")
                nc.sync.dma_start(zd[0:1, :], zrow[0:1, 0:NPC])
                nc.sync.dma_start(
                    v_t[0:55, :],
                    zd[0:1, :].rearrange("o (g n) -> (o n) g", n=N_PER))

            agg1T = gather_and_agg(1, yb1, f"g1_{r}")
            h2T = dense_update(1, h1T, agg1T, f"d1_{r}")

            # ======== layer 2 ========
            yb2 = u_and_allgather(2, h2T)
            agg2T = gather_and_agg(2, yb2, f"g2_{r}")
            h3T = dense_update(2, h2T, agg2T, f"d2_{r}")

            # ======== tail part B ========
            outb = sb2.tile([128, BPC * 2], F32, tag="outb", name=f"outb_{r}")
            if stage >= 5:
                xgrow = mlp3(h3T, f"xg{r}")
                nc.sync.dma_start(zd[1:2, :], xgrow[0:1, 0:NPC])
                nc.sync.dma_start(
                    v_t[55:110, :],
                    zd[1:2, :].rearrange("o (g n) -> (o n) g", n=N_PER))
                vb = sb2.tile([111, BPC], BF16, tag="vb", name=f"vb_{r}")
                nc.vector.tensor_copy(vb[:], v_t[:])
                for g in range(BPC):
                    for h in range(2):
                        col = g * 2 + h
                        pd = psu.tile([128, 1], F32, tag="u_ps",
                                      name=f"pdst_{r}_{col}")
                        nc.tensor.matmul(pd[:],
                                         m2_t[:, col * 128:(col + 1) * 128],
                                         vb[:, g:g + 1], start=True, stop=True)
                        nc.scalar.copy(outb[:, col:col + 1], pd[:])
            else:
                nc.vector.memset(outb[:], 0.0)
            nc.sync.dma_start(
                out_d.rearrange("g (h p) -> p g h", p=128),
                outb[:].rearrange("p (g h) -> p g h", h=2))

    nc.finalize()
    _cache[key] = nc
    return nc


def kernel(**inputs):
    coords = np.asarray(inputs["coords"], np.float32)
    src = np.asarray(inputs["src"])
    dst = np.asarray(inputs["dst"])
    destroy_ids = np.asarray(inputs["destroy_ids"])
    W_embed = np.asarray(inputs["W_embed"], np.float32)
    b_embed = np.asarray(inputs["b_embed"], np.float32)
    Wself = np.asarray(inputs["Wself"], np.float32)
    Wneigh = np.asarray(inputs["Wneigh"], np.float32)
    gnn_b = np.asarray(inputs["gnn_b"], np.float32)
    W1 = np.asarray(inputs["W1"], np.float32)
    b1 = np.asarray(inputs["b1"], np.float32)
    W2 = np.asarray(inputs["W2"], np.float32)
    b2 = np.asarray(inputs["b2"], np.float32)
    W3 = np.asarray(inputs["W3"], np.float32)
    b3 = np.asarray(inputs["b3"], np.float32)
    Wp = np.asarray(inputs["Wp"], np.float32)
    bp = np.asarray(inputs["bp"], np.float32)

    ntb, T, dloc_sb, src_wr, extras = _preprocess(
        coords, src, dst, destroy_ids, W_embed, b_embed, Wneigh, Wp, bp)
    coordsET, m2 = extras
    nc = _build(ntb, T, reps=1)

    waug = np.concatenate([W_embed, b_embed[None, :]], 0)          # [3, 64]
    w0p_np = (waug @ Wneigh[0]).astype(NPBF)
    iota128 = np.tile(np.arange(128, dtype=np.float32), (128, 1))

    in_maps = []
    for c in range(NC_):
        cs = coords[c * NPC:(c + 1) * NPC]                          # [1760, 2]
        coordsT = np.zeros((3, NPAD), np.float32)
        coordsT[:2, :NPC] = cs.T
        coordsT[2, :NPC] = 1.0
        in_maps.append({
            "coordsT": coordsT,
            "coordsET": coordsET[c],
            "w0p": w0p_np,
            "src_wr": src_wr[c],
            "dloc": dloc_sb[c],
            "m2": m2[c],
            "iota128": iota128,
            "waug": waug,
            "wself": np.ascontiguousarray(Wself.transpose(1, 0, 2)),
            "wneigh": np.ascontiguousarray(Wneigh.transpose(1, 0, 2)),
            "gnnbT": gnn_b.T.copy(),
            "w1": W1, "b1": b1[:, None], "w2": W2, "b2": b2[:, None],
            "w3": W3, "b3": b3[:, None],
            "bp": bp[:, None],
        })

    global _last_in_maps
    _last_in_maps = in_maps
    res = run_bass_kernel_spmd(nc, in_maps, core_ids=list(range(NC_)))
    return np.concatenate([res.results[c]["out"] for c in range(NC_)], 0)


# revision 14
# speedup vs baseline: 2.8767x; 1.4870x over previous
"""Trainium2 Bass kernel for nn_DestroyAgent (gnn_message_passing).

Math (algebraically reduced from the reference):
  h0 = coords @ W_embed + b_embed                      [N, 64]
  3 GNN layers: h = relu(h@Wself_l + segsum(h[src]@Wneigh_l, dst) + b_l) + h
  xg = MLP3(h3)[:, 0] per node;  z = MLP3(h0)[:, 0] per node
  out[b, k] = sum_n xg[b, n] * Wp[n] + sum_j z[b, ids[b,k,j]] * Wp[55+j] + bp

Sharding: 8 cores, core c owns graphs [32c, 32c+32) = nodes [1760c, 1760c+1760).
Edges are sharded by dst ownership, tiled into 128-edge tiles per 128-dst
block; aggregation = one-hot matmuls per tile (edge-contraction on PE).

Key structure vs the naive version:
- Layer 0 needs no gather/collective: u0[src] = coordsAug[src] @ (Waug@Wneigh0)
  and coordsAug[src] is host-gathered (static edge list), so the per-block
  aggregate is coordsET-tile @ onehot (PE) followed by a tiny [3,64] matmul.
- Layers 1-2: u = h@Wneigh rows are written to DRAM, AllGathered, then
  dma_gather'd per 128-edge tile (f32 256B rows), cast to bf16 for the
  one-hot matmuls.
- The destroy-set tail is a host-built matrix M2 [110, 128] per (graph, half):
  rows 0:55 = sum_j wp3[j]*onehot(ids), rows 55:110 = wp55; out column =
  M2^T @ concat(z_g, xg_g). 64 tiny matmuls replace the serial DVE loop.
- One-hot dst matrices are static across layers and reps: built once, bf16.
- Dense/MLP matmuls in float32r (1 cyc/row at free>=256 vs 4 for fp32).
"""

import numpy as np
from contextlib import ExitStack

import ml_dtypes
import concourse.bass as bass
import concourse.tile as tile
from concourse import bacc, mybir
from concourse.bass_utils import run_bass_kernel_spmd

dt = mybir.dt
F32 = dt.float32
F32R = dt.float32r
BF16 = dt.bfloat16
NPBF = ml_dtypes.bfloat16

B, K, D = 256, 256, 3
N_PER = 55
N = B * N_PER            # 14080
E = N * 8                # 112640
EMB = 64
NC_ = 8                  # cores
BPC = B // NC_           # graphs per core = 32
NPC = BPC * N_PER        # nodes per core = 1760
NPAD = 1792              # 14 * 128
NBLK = 14                # local 128-dst blocks
CHUNK = 448              # dense free-dim chunk (4 per core)
GCH = 8                  # gather chunks per layer

_cache = {}


def _preprocess(coords, src, dst, destroy_ids, W_embed=None, b_embed=None,
                Wneigh=None, Wp=None, bp=None):
    """Host-side index/static preprocessing -> per-core arrays + schedule."""
    order = np.argsort(dst, kind="stable")
    src_s = src[order].astype(np.int64)
    dst_s = dst[order].astype(np.int64)

    core_block_edges = []
    for c in range(NC_):
        lo_n = c * NPC
        blocks = []
        for b in range(NBLK):
            d0 = lo_n + 128 * b
            d1 = min(lo_n + 128 * (b + 1), lo_n + NPC)
            lo = np.searchsorted(dst_s, d0)
            hi = np.searchsorted(dst_s, d1)
            e_src = src_s[lo:hi]
            e_dst = dst_s[lo:hi]
            o = np.argsort(e_src, kind="stable")   # src-sorted for DMA locality
            blocks.append((e_src[o], e_dst[o] - d0))
        core_block_edges.append(blocks)

    ntb = [max(1, max((len(core_block_edges[c][b][0]) + 127) // 128
                      for c in range(NC_)))
           for b in range(NBLK)]
    T = sum(ntb)

    srcs, dlocs = [], []
    for c in range(NC_):
        s_parts, d_parts = [], []
        for b in range(NBLK):
            es, ed = core_block_edges[c][b]
            pad = ntb[b] * 128 - len(es)
            s_parts.append(np.concatenate([es, np.zeros(pad, np.int64)]))
            d_parts.append(np.concatenate([ed, np.full(pad, -1, np.int64)]))
        srcs.append(np.concatenate(s_parts))
        dlocs.append(np.concatenate(d_parts))
    src_all = np.stack(srcs)                     # [8, T*128]
    dloc_all = np.stack(dlocs)

    # dloc_sb [8, 128, T] fp32: edge t*128+p -> [p, t]
    dloc_sb = dloc_all.reshape(NC_, T, 128).transpose(0, 2, 1).astype(np.float32)
    # src idx wrap: idx i at [i%16, i//16], replicated x8 over partitions
    sw = src_all.reshape(NC_, T * 8, 16).transpose(0, 2, 1).astype(np.int16)
    src_wr = np.tile(sw, (1, 8, 1))              # [8, 128, T*8]

    extras = None
    if W_embed is not None:
        # per-edge augmented coords, tile-slot order: [8, 128(e), T*3] bf16
        caug = np.concatenate([coords, np.ones((N, 1), np.float32)], 1)  # [N,3]
        ce = caug[src_all]                                   # [8, T*128, 3]
        ce[dloc_all < 0] = 0.0
        coordsET = (ce.reshape(NC_, T, 128, 3).transpose(0, 2, 1, 3)
                    .reshape(NC_, 128, T * 3).astype(NPBF))

        # destroy matrix M2 [8, 110, BPC*2*128] bf16
        wp55 = Wp[:55, 0].astype(np.float32)
        wp3 = Wp[55:, 0].astype(np.float32)
        ids8 = destroy_ids.reshape(NC_, BPC, 2, 128, D)
        m2 = np.zeros((NC_, BPC, 2, 128, 111), np.float32)
        for j in range(D):
            np.add.at(m2, (np.arange(NC_)[:, None, None, None],
                           np.arange(BPC)[None, :, None, None],
                           np.arange(2)[None, None, :, None],
                           np.arange(128)[None, None, None, :],
                           ids8[..., j]), wp3[j])
        m2[..., 55:110] = wp55
        m2[..., 110] = float(bp[0])
        m2 = (m2.transpose(0, 4, 1, 2, 3).reshape(NC_, 111, BPC * 2 * 128)
              .astype(NPBF))
        extras = (coordsET, m2)

    return ntb, T, dloc_sb, src_wr, extras


def _build(ntb, T, reps=1, stage=99, nq=4):
    key = (tuple(ntb), T, reps, stage, nq)
    if key in _cache:
        return _cache[key]
    nc = bacc.Bacc("TRN2", target_bir_lowering=False, debug=False,
                   num_devices=NC_, num_swdge_queues=nq)

    def din(name, shape, dtype=F32):
        return nc.dram_tensor(name, list(shape), dtype, kind="ExternalInput").ap()

    coordsT_d = din("coordsT", [3, NPAD])
    coordsET_d = din("coordsET", [128, T * 3], BF16)
    w0p_d = din("w0p", [3, EMB], BF16)
    src_wr_d = din("src_wr", [128, T * 8], dt.int16)
    dloc_d = din("dloc", [128, T])
    m2_d = din("m2", [111, BPC * 2 * 128], BF16)
    iota128_d = din("iota128", [128, 128])
    waug_d = din("waug", [3, EMB])
    wself_d = din("wself", [EMB, 3, EMB])
    wneigh_d = din("wneigh", [EMB, 3, EMB])
    gnnbT_d = din("gnnbT", [EMB, 3])
    w1_d = din("w1", [EMB, 32])
    b1_d = din("b1", [32, 1])
    w2_d = din("w2", [32, 32])
    b2_d = din("b2", [32, 1])
    w3_d = din("w3", [32, 1])
    b3_d = din("b3", [1, 1])
    bp_d = din("bp", [1, 1])

    out_d = nc.dram_tensor("out", [BPC, K], F32, kind="ExternalOutput").ap()

    # DRAM scratch: u rows (local slice) + allgathered u, per rep, layers 1-2
    xbs = [[nc.dram_tensor(f"xb{r}_{l}", [NPC, EMB], F32) for l in range(2)]
           for r in range(reps)]
    ybs = [[nc.dram_tensor(f"yb{r}_{l}", [N, EMB], F32, addr_space="Shared")
            for l in range(2)] for r in range(reps)]
    # z/xg row bounce buffers (node-major -> [55, graphs] transpose via DRAM)
    zds = [nc.dram_tensor(f"zd{r}", [2, NPC], F32) for r in range(reps)]

    TPC = (T + GCH - 1) // GCH
    CHSZ = [min(TPC, T - g * TPC) for g in range(GCH) if T - g * TPC > 0]
    NCH = len(CHSZ)

    # global tile -> block schedule
    block_of = []
    for b in range(NBLK):
        block_of += [b] * ntb[b]
    first_t = {}
    last_t = {}
    for t, b in enumerate(block_of):
        if b not in first_t:
            first_t[b] = t
        last_t[b] = t

    with tile.TileContext(nc) as tc, ExitStack() as ctx:
        sb = ctx.enter_context(tc.tile_pool(name="sb", bufs=1))
        sb2 = ctx.enter_context(tc.tile_pool(name="sb2", bufs=2))
        sbg = ctx.enter_context(tc.tile_pool(name="sbg", bufs=2))
        ps = ctx.enter_context(tc.tile_pool(name="ps", bufs=2, space="PSUM"))
        psb = ctx.enter_context(tc.tile_pool(name="psb", bufs=3, space="PSUM"))
        psu = ctx.enter_context(tc.tile_pool(name="psu", bufs=2, space="PSUM"))

        def load(name, ap, shape, dtype=F32):
            t = sb.tile(list(shape), dtype, tag=name)
            nc.sync.dma_start(t[:], ap[:])
            return t

        coords_t = load("coordsT", coordsT_d, [3, NPAD])
        coordsET = load("coordsET", coordsET_d, [128, T * 3], BF16)
        w0p = load("w0p", w0p_d, [3, EMB], BF16)
        srcw_t = load("srcw", src_wr_d, [128, T * 8], dt.int16)
        dloc_t = load("dloct", dloc_d, [128, T])
        m2_t = load("m2", m2_d, [111, BPC * 2 * 128], BF16)
        iota128 = load("iota128t", iota128_d, [128, 128])
        waug = load("waugt", waug_d, [3, EMB])
        wself = load("wselft", wself_d, [EMB, 3, EMB])
        wneigh = load("wneight", wneigh_d, [EMB, 3, EMB])
        gnnbT = load("gnnbTt", gnnbT_d, [EMB, 3])
        w1 = load("w1t", w1_d, [EMB, 32])
        b1c = load("b1t", b1_d, [32, 1])
        w2 = load("w2t", w2_d, [32, 32])
        b2c = load("b2t", b2_d, [32, 1])
        w3 = load("w3t", w3_d, [32, 1])
        b3c = load("b3t", b3_d, [1, 1])
        bp_t = load("bpt", bp_d, [1, 1])

        # one-hot dst matrices: static across layers AND reps; bf16
        ohb = sb.tile([128, T, 128], BF16, tag="ohb")
        for t in range(T):
            nc.vector.tensor_scalar(ohb[:, t, :], iota128[:],
                                    dloc_t[:, t:t + 1], None,
                                    mybir.AluOpType.is_equal)

        def r32(apv):
            return apv

        def mlp3(srcT, nm):
            """64->32->32->1 MLP over all NPAD columns; returns [1, NPAD]."""
            row = sb2.tile([1, NPAD], F32, tag="mlprow", name=f"row_{nm}")
            for k in range(NPAD // CHUNK):
                sl = bass.ts(k, CHUNK)
                x1 = sb2.tile([32, CHUNK], F32, tag="mx1", name=f"x1_{nm}_{k}")
                x2 = sb2.tile([32, CHUNK], F32, tag="mx2", name=f"x2_{nm}_{k}")
                p1 = ps.tile([32, CHUNK], F32, tag="pre", name=f"p1_{nm}_{k}")
                nc.tensor.matmul(p1[:], r32(w1[:]), r32(srcT[:, sl]),
                                 start=True, stop=True)
                nc.scalar.activation(x1[:], p1[:],
                                     mybir.ActivationFunctionType.Lrelu,
                                     bias=b1c[:], alpha=0.01)
                p2 = ps.tile([32, CHUNK], F32, tag="pre", name=f"p2_{nm}_{k}")
                nc.tensor.matmul(p2[:], r32(w2[:]), r32(x1[:]),
                                 start=True, stop=True)
                nc.scalar.activation(x2[:], p2[:],
                                     mybir.ActivationFunctionType.Lrelu,
                                     bias=b2c[:], alpha=0.01)
                p3 = ps.tile([1, CHUNK], F32, tag="pre", name=f"p3_{nm}_{k}")
                nc.tensor.matmul(p3[:], r32(w3[:]), r32(x2[:]),
                                 start=True, stop=True)
                nc.scalar.activation(row[:, sl], p3[:],
                                     mybir.ActivationFunctionType.Identity,
                                     bias=b3c[0:1, 0:1])
            return row

        for r in range(reps):
            # ---- h0 = Waug.T @ coordsT ----
            h0T = sb2.tile([EMB, NPAD], F32, tag="h0T", name=f"h0T_{r}")
            for k in range(NPAD // CHUNK):
                p = ps.tile([EMB, CHUNK], F32, tag="pre", name=f"ph0_{r}_{k}")
                nc.tensor.matmul(p[:], r32(waug[:]),
                                 r32(coords_t[:, bass.ts(k, CHUNK)]),
                                 start=True, stop=True)
                nc.vector.tensor_copy(h0T[:, bass.ts(k, CHUNK)], p[:])

            def dense_update(l, hcur, aggT, nm):
                hnext = sb2.tile([EMB, NPAD], F32, tag=f"hh{l % 2}",
                                 name=f"h{l + 1}T_{r}")
                for k in range(NPAD // CHUNK):
                    sl = bass.ts(k, CHUNK)
                    p = ps.tile([EMB, CHUNK], F32, tag="pre",
                                name=f"pd_{nm}_{k}")
                    nc.tensor.matmul(p[:], r32(wself[:, l, :]),
                                     r32(hcur[:, sl]), start=True, stop=True)
                    nc.vector.tensor_add(p[:], p[:], aggT[:, sl])
                    relu = sb2.tile([EMB, CHUNK], F32, tag="relu",
                                    name=f"relu_{nm}_{k}")
                    nc.scalar.activation(relu[:], p[:],
                                         mybir.ActivationFunctionType.Relu,
                                         bias=gnnbT[:, l:l + 1])
                    nc.vector.tensor_add(hnext[:, sl], relu[:], hcur[:, sl])
                return hnext

            # ======== layer 0: agg from host-gathered edge coords ========
            agg0T = sb2.tile([EMB, NPAD], F32, tag="aggT", name=f"agg0T_{r}")
            if stage >= 4:
                cb_map = {}
                for t in range(T):
                    b = block_of[t]
                    if first_t[b] == t:
                        cb_map[b] = psu.tile([3, 128], F32, tag="u_ps",
                                             name=f"cb_{r}_{b}")
                    nc.tensor.matmul(cb_map[b][:],
                                     coordsET[:, t * 3:(t + 1) * 3],
                                     ohb[:, t, :],
                                     start=(first_t[b] == t),
                                     stop=(last_t[b] == t))
                    if last_t[b] == t:
                        cbs = sb2.tile([3, 128], BF16, tag="cbs",
                                       name=f"cbs_{r}_{b}")
                        nc.scalar.copy(cbs[:], cb_map.pop(b)[:])
                        pa = psu.tile([EMB, 128], F32, tag="u_ps",
                                      name=f"pa0_{r}_{b}")
                        nc.tensor.matmul(pa[:], w0p[:], cbs[:],
                                         start=True, stop=True)
                        nc.scalar.copy(agg0T[:, bass.ts(b, 128)], pa[:])
            else:
                nc.vector.memset(agg0T[:], 0.0)
            h1T = dense_update(0, h0T, agg0T, f"d0_{r}")

            def u_and_allgather(l, hcur):
                # u rows = h @ Wneigh_l (own slice), DMA out, AllGather
                u_sb = sb2.tile([128, NBLK, EMB], F32, tag="u_sb",
                                name=f"u_sb{r}_{l}")
                for t in range(NBLK):
                    pu = psu.tile([128, EMB], F32, tag="u_ps",
                                  name=f"pu{r}_{l}_{t}")
                    nc.tensor.matmul(pu[:], hcur[:, bass.ts(t, 128)],
                                     wneigh[:, l, :], start=True, stop=True)
                    nc.scalar.copy(u_sb[:, t, :], pu[:])
                xb, yb = xbs[r][l - 1].ap(), ybs[r][l - 1]
                nc.sync.dma_start(
                    xb[:1664].rearrange("(t p) f -> p t f", p=128),
                    u_sb[:, :13, :])
                nc.sync.dma_start(
                    xb[1664:].rearrange("(t p) f -> p t f", p=96),
                    u_sb[:96, 13:14, :])
                if stage >= 2:
                    nc.gpsimd.collective_compute(
                        "AllGather", mybir.AluOpType.bypass,
                        replica_groups=[list(range(NC_))],
                        ins=[xbs[r][l - 1].ap().opt()], outs=[yb.ap().opt()])
                return yb

            def gather_and_agg(l, yb, nm):
                aggT = sb2.tile([EMB, NPAD], F32, tag="aggT",
                                name=f"agg{l}T_{r}")
                if stage < 4:
                    nc.vector.memset(aggT[:], 0.0)
                pb_map = {}
                for g in range(NCH):
                    sz = CHSZ[g]
                    gt = sbg.tile([128, TPC, EMB], F32, tag="gath",
                                  name=f"gt{r}_{l}_{g}")
                    if stage >= 3:
                        nc.gpsimd.dma_gather(
                            gt[:, :sz, :], yb.ap(),
                            srcw_t[:, g * TPC * 8:(g * TPC + sz) * 8],
                            num_idxs=sz * 128, num_idxs_reg=sz * 128,
                            elem_size=EMB, queue_num=g % nq,
                            single_packet=(sz * 128 <= 1024))
                    else:
                        nc.vector.memset(gt[:], 0.0)
                    gtb = sbg.tile([128, TPC, EMB], BF16, tag="gathb",
                                   name=f"gtb{r}_{l}_{g}")
                    nc.vector.tensor_copy(gtb[:, :sz, :], gt[:, :sz, :])
                    if stage >= 4:
                        for k in range(sz):
                            t = g * TPC + k
                            b = block_of[t]
                            if first_t[b] == t:
                                pb_map[b] = psb.tile(
                                    [EMB, 128], F32, tag="blk",
                                    name=f"pbm{r}_{l}_{b}")
                            nc.tensor.matmul(pb_map[b][:], gtb[:, k, :],
                                             ohb[:, t, :],
                                             start=(first_t[b] == t),
                                             stop=(last_t[b] == t))
                            if last_t[b] == t:
                                nc.scalar.copy(aggT[:, bass.ts(b, 128)],
                                               pb_map.pop(b)[:])
                return aggT

            # ======== layer 1 ========
            yb1 = u_and_allgather(1, h1T)

            # tail part A (z-mlp from h0) fills the AllGather window
            v_t = sb2.tile([111, BPC], F32, tag="v", name=f"v_{r}")
            nc.vector.memset(v_t[:], 1.0)
            zd = zds[r].ap()
            if stage >= 5:
                zrow = mlp3(h0T, f"z{r}")
                nc.sync.dma_start(zd[0:1, :], zrow[0:1, 0:NPC])
                nc.sync.dma_start(
                    v_t[0:55, :],
                    zd[0:1, :].rearrange("o (g n) -> (o n) g", n=N_PER))

            agg1T = gather_and_agg(1, yb1, f"g1_{r}")
            h2T = dense_update(1, h1T, agg1T, f"d1_{r}")

            # ======== layer 2 ========
            yb2 = u_and_allgather(2, h2T)
            agg2T = gather_and_agg(2, yb2, f"g2_{r}")
            h3T = dense_update(2, h2T, agg2T, f"d2_{r}")

            # ======== tail part B ========
            outb = sb2.tile([128, BPC * 2], F32, tag="outb", name=f"outb_{r}")
            if stage >= 5:
                xgrow = mlp3(h3T, f"xg{r}")
                nc.sync.dma_start(zd[1:2, :], xgrow[0:1, 0:NPC])
                nc.sync.dma_start(
                    v_t[55:110, :],
                    zd[1:2, :].rearrange("o (g n) -> (o n) g", n=N_PER))
                vb = sb2.tile([111, BPC], BF16, tag="vb", name=f"vb_{r}")
                nc.vector.tensor_copy(vb[:], v_t[:])
                for g in range(BPC):
                    for h in range(2):
                        col = g * 2 + h
                        pd = psu.tile([128, 1], F32, tag="u_ps",
                                      name=f"pdst_{r}_{col}")
                        nc.tensor.matmul(pd[:],
                                         m2_t[:, col * 128:(col + 1) * 128],
                                         vb[:, g:g + 1], start=True, stop=True)
                        nc.scalar.copy(outb[:, col:col + 1], pd[:])
            else:
                nc.vector.memset(outb[:], 0.0)
            nc.sync.dma_start(
                out_d.rearrange("g (h p) -> p g h", p=128),
                outb[:].rearrange("p (g h) -> p g h", h=2))

    nc.finalize()
    _cache[key] = nc
    return nc


def kernel(**inputs):
    coords = np.asarray(inputs["coords"], np.float32)
    src = np.asarray(inputs["src"])
    dst = np.asarray(inputs["dst"])
    destroy_ids = np.asarray(inputs["destroy_ids"])
    W_embed = np.asarray(inputs["W_embed"], np.float32)
    b_embed = np.asarray(inputs["b_embed"], np.float32)
    Wself = np.asarray(inputs["Wself"], np.float32)
    Wneigh = np.asarray(inputs["Wneigh"], np.float32)
    gnn_b = np.asarray(inputs["gnn_b"], np.float32)
    W1 = np.asarray(inputs["W1"], np.float32)
    b1 = np.asarray(inputs["b1"], np.float32)
    W2 = np.asarray(inputs["W2"], np.float32)
    b2 = np.asarray(inputs["b2"], np.float32)
    W3 = np.asarray(inputs["W3"], np.float32)
    b3 = np.asarray(inputs["b3"], np.float32)
    Wp = np.asarray(inputs["Wp"], np.float32)
    bp = np.asarray(inputs["bp"], np.float32)

    ntb, T, dloc_sb, src_wr, extras = _preprocess(
        coords, src, dst, destroy_ids, W_embed, b_embed, Wneigh, Wp, bp)
    coordsET, m2 = extras
    nc = _build(ntb, T, reps=1)

    waug = np.concatenate([W_embed, b_embed[None, :]], 0)          # [3, 64]
    w0p_np = (waug @ Wneigh[0]).astype(NPBF)
    iota128 = np.tile(np.arange(128, dtype=np.float32), (128, 1))

    in_maps = []
    for c in range(NC_):
        cs = coords[c * NPC:(c + 1) * NPC]                          # [1760, 2]
        coordsT = np.zeros((3, NPAD), np.float32)
        coordsT[:2, :NPC] = cs.T
        coordsT[2, :NPC] = 1.0
        in_maps.append({
            "coordsT": coordsT,
            "coordsET": coordsET[c],
            "w0p": w0p_np,
            "src_wr": src_wr[c],
            "dloc": dloc_sb[c],
            "m2": m2[c],
            "iota128": iota128,
            "waug": waug,
            "wself": np.ascontiguousarray(Wself.transpose(1, 0, 2)),
            "wneigh": np.ascontiguousarray(Wneigh.transpose(1, 0, 2)),
            "gnnbT": gnn_b.T.copy(),
            "w1": W1, "b1": b1[:, None], "w2": W2, "b2": b2[:, None],
            "w3": W3, "b3": b3[:, None],
            "bp": bp[:, None],
        })

    global _last_in_maps
    _last_in_maps = in_maps
    res = run_bass_kernel_spmd(nc, in_maps, core_ids=list(range(NC_)))
    return np.concatenate([res.results[c]["out"] for c in range(NC_)], 0)
